# revision 7
# baseline (speedup 1.0000x reference)
"""DoRA Linear on 8 Trainium2 NeuronCores (Bass/Tile).

Reference computation (all fp32):
    new_v   = base_weight + SCALE * dora_B @ dora_A          [OUT, IN]
    scale_o = weight_m / ||new_v||_row                        [OUT]
    out     = x @ (scale_o[:, None] * new_v)^T + base_bias    [B, S, OUT]

Sharding: column-parallel over OUT across 8 cores (OUT/8 = 512 each).
base_weight, dora_B, weight_m, base_bias sharded; x, dora_A replicated.

Per-core device program (heavy math in float32r on the PE):
  1. Build W'^T = (W + SCALE*B@A)^T chunk-by-chunk in SBUF:
     PE matmul A^T@(2B^T) -> PSUM, DVE adds the W^T chunk and writes the
     f32r weight tile `wr` the main matmul consumes.
  2. Row norms: sq = wr*wr (DVE), then PE matmuls with sq as lhsT and an
     all-ones rhs (N=256) accumulate sum_i sq[i, o] over k-chunks,
     giving norms directly in o-partition column layout.  NOTE: lhsT
     must differ between consecutive f32r matmuls - walrus turns
     repeated identical weights into a non-self-loading matmul, which
     produces a NEFF the runtime refuses to load for f32/f32r.
  3. scale_col = weight_m / sqrt(norm2)  (ACT sqrt + DVE reciprocal/mul)
  4. Main matmul, output-transposed orientation:
     outT[o, m] = sum_k W'^T[k, o] * xT[k, m], PSUM-accumulated over 32
     k-chunks; eviction fuses *scale_o + bias_o in one DVE tensor_scalar
     (per-partition scalars).
Host: layout transposes in numpy (no FLOPs), final gather + transpose.
"""

import numpy as np

import concourse.bass as bass
import concourse.mybir as mybir
import concourse.tile as tile
from concourse import bacc
from concourse.bass_utils import run_bass_kernel_spmd

OUT, IN, RANK = 4096, 4096, 16
SCALE = 2.0
NCORES = 8
OSH = OUT // NCORES          # 512 out features per core
P = 128
KO = IN // P                 # 32 k-chunks
KQ = 4                       # k-quarters for x streaming
KO_Q = KO // KQ              # 8 k-chunks per x tile
M = 4 * 2048                 # 8192 tokens
MCH = 512                    # tokens per x tile
NM = M // MCH                # 16 m-chunks
OC = OSH // P                # 4 o-chunks of 128
NRW = 256                    # ones rhs width for norm matmuls (f32r min)

F32 = mybir.dt.float32
F32R = mybir.dt.float32r


def _build():
    nc = bacc.Bacc(None, target_bir_lowering=False)
    xT = nc.dram_tensor("xT", [P, KO, M], F32R, kind="ExternalInput")
    wT = nc.dram_tensor("wT", [P, KO, OSH], F32, kind="ExternalInput")
    aT = nc.dram_tensor("aT", [RANK, IN], F32R, kind="ExternalInput")
    bT = nc.dram_tensor("bT", [RANK, OSH], F32, kind="ExternalInput")
    wm = nc.dram_tensor("wm", [P, OC], F32, kind="ExternalInput")
    bc = nc.dram_tensor("bc", [P, OC], F32, kind="ExternalInput")
    outT = nc.dram_tensor("outT", [OSH, M], F32, kind="ExternalOutput")
    outT_v = outT.ap().rearrange("(oc p) m -> oc p m", p=P)

    with tile.TileContext(nc) as tc:
        with (
            tc.tile_pool(name="wr", bufs=1) as wrpool,
            tc.tile_pool(name="const", bufs=1) as cpool,
            tc.tile_pool(name="wv", bufs=2) as wvpool,
            tc.tile_pool(name="ach", bufs=2) as apool,
            tc.tile_pool(name="sq", bufs=2) as sqpool,
            tc.tile_pool(name="xs", bufs=4) as xpool,
            tc.tile_pool(name="os", bufs=3) as opool,
            tc.tile_pool(name="ps_nr", bufs=1, space="PSUM") as ps_nr,
            tc.tile_pool(name="ps_mm", bufs=6, space="PSUM") as ps_mm,
        ):
            # ---- constants ----
            bt_f = cpool.tile([RANK, OSH], F32)
            nc.sync.dma_start(bt_f[:], bT.ap())
            bt2 = cpool.tile([RANK, OSH], F32R)
            nc.vector.tensor_scalar_mul(bt2[:], bt_f[:], SCALE)
            ones_f = cpool.tile([P, NRW], F32)
            nc.any.memset(ones_f[:], 1.0)
            ones_r = cpool.tile([P, NRW], F32R)
            nc.vector.tensor_copy(ones_r[:], ones_f[:])
            wm_col = cpool.tile([P, OC], F32)
            nc.sync.dma_start(wm_col[:], wm.ap())
            bias_col = cpool.tile([P, OC], F32)
            nc.sync.dma_start(bias_col[:], bc.ap())

            # ---- DoRA weight prep + row-norm accumulation ----
            wr = wrpool.tile([P, KO, OSH], F32R)
            nrc = ps_nr.tile([P, OC * NRW], F32)   # 2 PSUM banks
            for kq in range(KQ):
                wv = wvpool.tile([P, KO_Q, OSH], F32)
                nc.sync.dma_start(wv[:], wT.ap()[:, kq * KO_Q:(kq + 1) * KO_Q])
                ach = apool.tile([RANK, KO_Q * P], F32R)
                nc.sync.dma_start(
                    ach[:], aT.ap()[:, kq * KO_Q * P:(kq + 1) * KO_Q * P])
                for k8 in range(KO_Q):
                    ko = kq * KO_Q + k8
                    ba = ps_mm.tile([P, OSH], F32, name="mm")
                    nc.tensor.matmul(
                        ba[:], ach[:, k8 * P:(k8 + 1) * P], bt2[:],
                        start=True, stop=True)
                    nc.vector.tensor_tensor(
                        wr[:, ko], wv[:, k8], ba[:], mybir.AluOpType.add)
                    sq = sqpool.tile([P, OSH], F32R)
                    nc.vector.tensor_tensor(
                        sq[:], wr[:, ko], wr[:, ko], mybir.AluOpType.mult)
                    for oc in range(OC):
                        nc.tensor.matmul(
                            nrc[:, oc * NRW:(oc + 1) * NRW],
                            sq[:, oc * P:(oc + 1) * P], ones_r[:],
                            start=(ko == 0), stop=(ko == KO - 1))

            # ---- scale_col = wm / sqrt(norm2) in o-partition layout ----
            sqc = cpool.tile([P, OC], F32)
            for oc in range(OC):
                nc.scalar.activation(
                    sqc[:, oc:oc + 1], nrc[:, oc * NRW:oc * NRW + 1],
                    mybir.ActivationFunctionType.Sqrt)
            rcp = cpool.tile([P, OC], F32)
            nc.vector.reciprocal(rcp[:], sqc[:])
            scale_col = cpool.tile([P, OC], F32)
            nc.vector.tensor_tensor(
                scale_col[:], wm_col[:], rcp[:], mybir.AluOpType.mult)

            # ---- main matmul: outT[o, m] accumulated over k ----
            for mc in range(NM):
                pss = [ps_mm.tile([P, MCH], F32, name="mm")
                       for _ in range(OC)]
                for kq in range(KQ):
                    xt = xpool.tile([P, KO_Q, MCH], F32R)
                    nc.sync.dma_start(
                        xt[:],
                        xT.ap()[:, kq * KO_Q:(kq + 1) * KO_Q,
                                mc * MCH:(mc + 1) * MCH])
                    for oc in range(OC):
                        for k8 in range(KO_Q):
                            nc.tensor.matmul(
                                pss[oc][:],
                                wr[:, kq * KO_Q + k8, oc * P:(oc + 1) * P],
                                xt[:, k8],
                                start=(kq == 0 and k8 == 0),
                                stop=(kq == KQ - 1 and k8 == KO_Q - 1))
                for oc in range(OC):
                    ot = opool.tile([P, MCH], F32)
                    nc.vector.tensor_scalar(
                        ot[:], pss[oc][:],
                        scale_col[:, oc:oc + 1], bias_col[:, oc:oc + 1],
                        mybir.AluOpType.mult, mybir.AluOpType.add)
                    nc.sync.dma_start(
                        outT_v[oc, :, mc * MCH:(mc + 1) * MCH], ot[:])
    nc.compile()
    return nc


def kernel(x, base_weight, base_bias, weight_m, dora_A, dora_B):
    x = np.asarray(x, dtype=np.float32)
    base_weight = np.asarray(base_weight, dtype=np.float32)
    base_bias = np.asarray(base_bias, dtype=np.float32)
    weight_m = np.asarray(weight_m, dtype=np.float32)
    dora_A = np.asarray(dora_A, dtype=np.float32)
    dora_B = np.asarray(dora_B, dtype=np.float32)

    B, S, _ = x.shape
    assert B * S == M and x.shape[2] == IN

    # xT[p, ko, m] = x[m, ko*128+p]  (shared across all cores)
    x2 = x.reshape(M, KO, P)
    xT = np.ascontiguousarray(x2.transpose(2, 1, 0))

    in_maps = []
    for c in range(NCORES):
        sl = slice(c * OSH, (c + 1) * OSH)
        w_c = base_weight[sl]                                   # [OSH, IN]
        wT_c = np.ascontiguousarray(
            w_c.reshape(OSH, KO, P).transpose(2, 1, 0))         # [P, KO, OSH]
        bT_c = np.ascontiguousarray(dora_B[sl].T)               # [RANK, OSH]
        wm_c = np.ascontiguousarray(weight_m[sl].reshape(OC, P).T)
        bc_c = np.ascontiguousarray(base_bias[sl].reshape(OC, P).T)
        in_maps.append({
            "xT": xT,
            "wT": wT_c,
            "aT": dora_A,
            "bT": bT_c,
            "wm": wm_c,
            "bc": bc_c,
        })

    nc = _build()
    res = run_bass_kernel_spmd(nc, in_maps, core_ids=list(range(NCORES)))

    full = np.empty((OUT, M), dtype=np.float32)
    for c in range(NCORES):
        full[c * OSH:(c + 1) * OSH] = res.results[c]["outT"]
    return np.ascontiguousarray(full.T).reshape(B, S, OUT)


# revision 9
# speedup vs baseline: 1.0570x; 1.0570x over previous
"""DoRA Linear on 8 Trainium2 NeuronCores (Bass/Tile).

Reference computation (all fp32):
    new_v   = base_weight + SCALE * dora_B @ dora_A          [OUT, IN]
    scale_o = weight_m / ||new_v||_row                        [OUT]
    out     = x @ (scale_o[:, None] * new_v)^T + base_bias    [B, S, OUT]

Sharding: column-parallel over OUT across 8 cores (OUT/8 = 512 each).
base_weight, dora_B, weight_m, base_bias sharded; x, dora_A replicated.

Per-core device program (heavy math in float32r on the PE):
  1. Build W'^T = (W + SCALE*B@A)^T chunk-by-chunk in SBUF:
     PE matmul A^T@(2B^T) -> PSUM, DVE adds the W^T chunk and writes the
     f32r weight tile `wr` the main matmul consumes.
  2. Row norms: sq = wr*wr (DVE), then PE matmuls with sq as lhsT and an
     all-ones rhs (N=256) accumulate sum_i sq[i, o] over k-chunks,
     giving norms directly in o-partition column layout.  NOTE: lhsT
     must differ between consecutive f32r matmuls - walrus turns
     repeated identical weights into a non-self-loading matmul, which
     produces a NEFF the runtime refuses to load for f32/f32r.
  3. scale_col = weight_m / sqrt(norm2)  (ACT sqrt + DVE reciprocal/mul)
  4. Main matmul, output-transposed orientation:
     outT[o, m] = sum_k W'^T[k, o] * xT[k, m], PSUM-accumulated over 32
     k-chunks; eviction fuses *scale_o + bias_o in one DVE tensor_scalar
     (per-partition scalars).
Host: layout transposes in numpy (no FLOPs), final gather + transpose.
"""

import numpy as np

import concourse.bass as bass
import concourse.mybir as mybir
import concourse.tile as tile
from concourse import bacc
from concourse.bass_utils import run_bass_kernel_spmd

OUT, IN, RANK = 4096, 4096, 16
SCALE = 2.0
NCORES = 8
OSH = OUT // NCORES          # 512 out features per core
P = 128
KO = IN // P                 # 32 k-chunks
KQ = 4                       # k-quarters for x streaming
KO_Q = KO // KQ              # 8 k-chunks per x tile
M = 4 * 2048                 # 8192 tokens
MCH = 512                    # tokens per x tile
NM = M // MCH                # 16 m-chunks
OC = OSH // P                # 4 o-chunks of 128
NRW = 256                    # ones rhs width for norm matmuls (f32r min)

F32 = mybir.dt.float32
F32R = mybir.dt.float32r


def _build():
    nc = bacc.Bacc(None, target_bir_lowering=False)
    xT = nc.dram_tensor("xT", [P, KO, M], F32R, kind="ExternalInput")
    wT = nc.dram_tensor("wT", [P, KO, OSH], F32, kind="ExternalInput")
    aT = nc.dram_tensor("aT", [RANK, IN], F32R, kind="ExternalInput")
    bT = nc.dram_tensor("bT", [RANK, OSH], F32, kind="ExternalInput")
    wm = nc.dram_tensor("wm", [P, OC], F32, kind="ExternalInput")
    bc = nc.dram_tensor("bc", [P, OC], F32, kind="ExternalInput")
    outT = nc.dram_tensor("outT", [OSH, M], F32, kind="ExternalOutput")
    outT_v = outT.ap().rearrange("(oc p) m -> oc p m", p=P)

    with tile.TileContext(nc) as tc:
        with (
            tc.tile_pool(name="wr", bufs=1) as wrpool,
            tc.tile_pool(name="const", bufs=1) as cpool,
            tc.tile_pool(name="wv", bufs=2) as wvpool,
            tc.tile_pool(name="ach", bufs=2) as apool,
            tc.tile_pool(name="sq", bufs=2) as sqpool,
            tc.tile_pool(name="xs", bufs=4) as xpool,
            tc.tile_pool(name="os", bufs=3) as opool,
            tc.tile_pool(name="ps_nr", bufs=2, space="PSUM") as ps_nr,
            tc.tile_pool(name="ps_mm", bufs=6, space="PSUM") as ps_mm,
        ):
            # ---- constants ----
            bt_f = cpool.tile([RANK, OSH], F32)
            nc.sync.dma_start(bt_f[:], bT.ap())
            bt2 = cpool.tile([RANK, OSH], F32R)
            nc.vector.tensor_scalar_mul(bt2[:], bt_f[:], SCALE)
            ones_f = cpool.tile([P, NRW], F32)
            nc.any.memset(ones_f[:], 1.0)
            ones_r = cpool.tile([P, NRW], F32R)
            nc.vector.tensor_copy(ones_r[:], ones_f[:])
            wm_col = cpool.tile([P, OC], F32)
            nc.sync.dma_start(wm_col[:], wm.ap())
            bias_col = cpool.tile([P, OC], F32)
            nc.sync.dma_start(bias_col[:], bc.ap())

            # ---- DoRA weight prep: wr = (W + 2 B A)^T in f32r ----
            wr = wrpool.tile([P, KO, OSH], F32R)
            for kq in range(KQ):
                wv = wvpool.tile([P, KO_Q, OSH], F32)
                nc.sync.dma_start(wv[:], wT.ap()[:, kq * KO_Q:(kq + 1) * KO_Q])
                ach = apool.tile([RANK, KO_Q * P], F32R)
                nc.sync.dma_start(
                    ach[:], aT.ap()[:, kq * KO_Q * P:(kq + 1) * KO_Q * P])
                for k8 in range(KO_Q):
                    ko = kq * KO_Q + k8
                    ba = ps_mm.tile([P, OSH], F32, name="mm")
                    nc.tensor.matmul(
                        ba[:], ach[:, k8 * P:(k8 + 1) * P], bt2[:],
                        start=True, stop=True)
                    nc.vector.tensor_tensor(
                        wr[:, ko], wv[:, k8], ba[:], mybir.AluOpType.add)

            # ---- row norms, one o-chunk (= one accumulation group, one
            # PSUM bank) at a time: concurrent groups must not share a
            # bank, start=True clears whole-bank has_written state ----
            sqc = cpool.tile([P, OC], F32)
            for oc in range(OC):
                nrc = ps_nr.tile([P, NRW], F32, name="nrc")
                for ko in range(KO):
                    sq = sqpool.tile([P, P], F32R)
                    nc.vector.tensor_tensor(
                        sq[:], wr[:, ko, oc * P:(oc + 1) * P],
                        wr[:, ko, oc * P:(oc + 1) * P], mybir.AluOpType.mult)
                    nc.tensor.matmul(
                        nrc[:], sq[:], ones_r[:],
                        start=(ko == 0), stop=(ko == KO - 1))
                nc.scalar.activation(
                    sqc[:, oc:oc + 1], nrc[:, 0:1],
                    mybir.ActivationFunctionType.Sqrt)
            rcp = cpool.tile([P, OC], F32)
            nc.vector.reciprocal(rcp[:], sqc[:])
            scale_col = cpool.tile([P, OC], F32)
            nc.vector.tensor_tensor(
                scale_col[:], wm_col[:], rcp[:], mybir.AluOpType.mult)

            # ---- main matmul: outT[o, m] accumulated over k ----
            for mc in range(NM):
                pss = [ps_mm.tile([P, MCH], F32, name="mm")
                       for _ in range(OC)]
                for kq in range(KQ):
                    xt = xpool.tile([P, KO_Q, MCH], F32R)
                    nc.sync.dma_start(
                        xt[:],
                        xT.ap()[:, kq * KO_Q:(kq + 1) * KO_Q,
                                mc * MCH:(mc + 1) * MCH])
                    for oc in range(OC):
                        for k8 in range(KO_Q):
                            nc.tensor.matmul(
                                pss[oc][:],
                                wr[:, kq * KO_Q + k8, oc * P:(oc + 1) * P],
                                xt[:, k8],
                                start=(kq == 0 and k8 == 0),
                                stop=(kq == KQ - 1 and k8 == KO_Q - 1))
                for oc in range(OC):
                    ot = opool.tile([P, MCH], F32)
                    nc.vector.tensor_scalar(
                        ot[:], pss[oc][:],
                        scale_col[:, oc:oc + 1], bias_col[:, oc:oc + 1],
                        mybir.AluOpType.mult, mybir.AluOpType.add)
                    nc.sync.dma_start(
                        outT_v[oc, :, mc * MCH:(mc + 1) * MCH], ot[:])
    nc.compile()
    return nc


def kernel(x, base_weight, base_bias, weight_m, dora_A, dora_B):
    x = np.asarray(x, dtype=np.float32)
    base_weight = np.asarray(base_weight, dtype=np.float32)
    base_bias = np.asarray(base_bias, dtype=np.float32)
    weight_m = np.asarray(weight_m, dtype=np.float32)
    dora_A = np.asarray(dora_A, dtype=np.float32)
    dora_B = np.asarray(dora_B, dtype=np.float32)

    B, S, _ = x.shape
    assert B * S == M and x.shape[2] == IN

    # xT[p, ko, m] = x[m, ko*128+p]  (shared across all cores)
    x2 = x.reshape(M, KO, P)
    xT = np.ascontiguousarray(x2.transpose(2, 1, 0))

    in_maps = []
    for c in range(NCORES):
        sl = slice(c * OSH, (c + 1) * OSH)
        w_c = base_weight[sl]                                   # [OSH, IN]
        wT_c = np.ascontiguousarray(
            w_c.reshape(OSH, KO, P).transpose(2, 1, 0))         # [P, KO, OSH]
        bT_c = np.ascontiguousarray(dora_B[sl].T)               # [RANK, OSH]
        wm_c = np.ascontiguousarray(weight_m[sl].reshape(OC, P).T)
        bc_c = np.ascontiguousarray(base_bias[sl].reshape(OC, P).T)
        in_maps.append({
            "xT": xT,
            "wT": wT_c,
            "aT": dora_A,
            "bT": bT_c,
            "wm": wm_c,
            "bc": bc_c,
        })

    nc = _build()
    res = run_bass_kernel_spmd(nc, in_maps, core_ids=list(range(NCORES)))

    full = np.empty((OUT, M), dtype=np.float32)
    for c in range(NCORES):
        full[c * OSH:(c + 1) * OSH] = res.results[c]["outT"]
    return np.ascontiguousarray(full.T).reshape(B, S, OUT)
